# revision 7
# baseline (speedup 1.0000x reference)
"""Trainium2 Bass kernel for nn_BackProjNet (segment_reduce), triple-row table.

out[c, v] = (sum_r x[c, idx[v, r]] * w[v, r]) * SCALE + bias[v]

Strategy (8 NeuronCores, voxel-sharded):
  - Each core owns 8192 voxels (voxel v -> partition p = v//64, layer jj = v%64).
  - The sinogram table packs THREE positions per 256B row (row j holds
    x[:, 3j], x[:, 3j+1], x[:, 3j+2] as 24 bf16 = 48B payload), so all
    92160 positions fit in 30720 rows and gather indices stay int16 with
    NO position coloring and NO per-(voxel,color) padding: every voxel is
    exactly 360 gather slots (4.8% fewer SWDGE indices than the colored
    layout, and the kernel is descriptor-generation bound).
  - Gathers run as InstDMAGatherAnt (48B payload, 256B row stride), 1024
    indices per instruction on 4 SWDGE queues, 45 instructions per
    voxel-layer stage.
  - DVE multiplies by a host-packed masked weight stream (w at the ray's
    sub-position, 0 elsewhere, broadcast over the 8 channels) and reduces
    the 360x3 slot-sub grid per voxel; bias is added once at the end.
"""

import os
import sys

import numpy as np
import ml_dtypes

for _p in ("/opt/trn_rl_repo", "/root/.axon_site/_ro/trn_rl_repo"):
    if _p not in sys.path:
        sys.path.append(_p)

import concourse.bass as bass
import concourse.bacc as bacc
import concourse.mybir as mybir
import concourse.tile as tile
from concourse import ap_utils
from concourse._compat import exact_div
from concourse.bass import round_up_to_multiple
from concourse.bass_interp import get_hw_module

# geometry (must match reference.py)
CHANNEL = 8
NVX, NVY = 256, 256
VIEWS, EXTENT = 180, 2
NDETU = 512
SCALE = (2.0 * np.pi - 0.0) / (2.0 * VIEWS * EXTENT)

NCORES = 8
P = 128
V = VIEWS * NDETU          # 92160 sinogram positions
R = VIEWS * EXTENT         # 360 rays per voxel
NVOX = NVX * NVY           # 65536 voxels
VPC = NVOX // NCORES       # 8192 voxels per core
VPP = VPC // P             # 64 voxels per partition ("stages")
SUB = 3                    # positions per table row
ROWS = V // SUB            # 30720 table rows (int16-safe)
U = 3                      # sub slots per gathered row
E = U * CHANNEL            # 24 payload values per row (48B)
PITCH = 128                # bf16 per table row (256B)
NPI = 1024                 # indices per gather instruction (ucode limit)
SPI = NPI // P             # 8 slots per partition per instruction
NINST = R // SPI           # 45 gather instructions per stage
NQUEUES = int(os.environ.get("KQ", "4"))


def _dma_gather_raw(gpsimd, out_ap, in_ap, idxs_ap, num_idxs, elem_size,
                    elem_step, queue_num):
    """bass.dma_gather without the elem_size%256 restriction (the 256B
    constraint is on the row stride, which we satisfy with PITCH=128 bf16)."""
    self = gpsimd
    assert idxs_ap.dtype == mybir.dt.int16
    assert in_ap.space == bass.MemorySpace.DRAM
    assert in_ap.dtype == out_ap.dtype
    assert idxs_ap.space == bass.MemorySpace.SBUF
    assert out_ap.space == bass.MemorySpace.SBUF
    assert ap_utils.ap_is_contiguous(out_ap.ap[1:])
    assert ap_utils.ap_is_contiguous(idxs_ap.ap[1:])
    assert in_ap.ap[-1][1] == out_ap.ap[-1][1] == elem_size
    assert out_ap.ap[0][1] * out_ap.ap[1][1] == round_up_to_multiple(num_idxs, 128)
    assert in_ap.ap[0][0] == elem_step
    stride_bytes = elem_step * mybir.dt.size(in_ap.dtype)
    stride_bytes_256 = exact_div(stride_bytes, 256)
    _in_ap = self.lower_ap_dma(in_ap, for_custom_bir_dma=True)
    _idxs_ap = self.lower_ap(idxs_ap)
    _out_ap = self.lower_ap(out_ap)
    return self.add_instruction(
        mybir.InstDMAGatherAnt(
            name=self.bass.get_next_instruction_name(),
            ins=[*_in_ap, _idxs_ap, self.lower_val_access(self.to_reg(num_idxs))],
            outs=[_out_ap],
            transpose=False,
            num_idxs=num_idxs,
            elem_size=elem_size,
            stride_bytes_256=stride_bytes_256,
            gen_mode=0,
            single_packet=bool(int(os.environ.get("KSP", "0"))),
            queue_num=queue_num,
            sbuf_tokens_per_rank=0,
            sbuf_free_dim_per_rank=0,
            sbuf_free_dim_pad_per_rank=0,
            sbuf_byte_offset=0,
        )
    )


# ---------------------------------------------------------------- packing

def _prep_core(x_tab, w2, idx2, bias_m):
    """Build one core's device arrays.

    x_tab: shared table [ROWS, PITCH] bf16.  idx2/w2: [VPC, R].
    """
    rows = (idx2 // SUB).astype(np.int16)             # [VPC, R]
    subs = (idx2 % SUB).astype(np.int64)

    # per-partition voxel-major slot stream: partition p = voxel // VPP
    arr = rows.reshape(P, VPP, NINST, SPI)            # [p, stage, n, q]
    # instruction idx list i = q*128 + p, wrapped over 16 partitions
    a = arr.transpose(1, 2, 3, 0).reshape(VPP, NINST, NPI)
    idx_dev = np.ascontiguousarray(
        a.reshape(VPP, NINST, NPI // 16, 16).transpose(0, 1, 3, 2)
        .transpose(0, 2, 1, 3).reshape(VPP, 16, NINST * (NPI // 16)))

    w24 = np.zeros((VPC, R, U), np.float32)
    vv = np.repeat(np.arange(VPC), R)
    rr = np.tile(np.arange(R), VPC)
    w24[vv, rr, subs.ravel()] = (w2 * np.float32(SCALE)).ravel()
    w_dev = np.ascontiguousarray(
        w24.reshape(P, VPP, R * U).transpose(1, 0, 2)).astype(
            ml_dtypes.bfloat16)                       # [VPP, P, 1440]

    bias_dev = np.ascontiguousarray(
        np.repeat(bias_m.reshape(P, VPP), CHANNEL).reshape(P, VPP * CHANNEL))

    return dict(tab=x_tab, idx=idx_dev, wts=w_dev, biasx=bias_dev)


# ---------------------------------------------------------------- module

def _build_module(hw=True):
    nc = bacc.Bacc(
        "TRN2",
        target_bir_lowering=False,
        debug=False,
        num_devices=NCORES,
        dynamic_dma_scratch_size=16384,
        num_swdge_queues=NQUEUES,
    )
    tab_d = nc.dram_tensor("tab", [ROWS, PITCH], mybir.dt.bfloat16,
                           kind="ExternalInput")
    idx_d = nc.dram_tensor("idx", [VPP, 16, NINST * (NPI // 16)],
                           mybir.dt.int16, kind="ExternalInput")
    w_d = nc.dram_tensor("wts", [VPP, P, R * U], mybir.dt.bfloat16,
                         kind="ExternalInput")
    b_d = nc.dram_tensor("biasx", [P, VPP * CHANNEL], mybir.dt.float32,
                         kind="ExternalInput")
    out_d = nc.dram_tensor("out", [P, VPP * CHANNEL], mybir.dt.float32,
                           kind="ExternalOutput")

    tab_ap = tab_d.ap()
    idx_ap = idx_d.ap()
    w_ap = w_d.ap()

    with tile.TileContext(nc) as tc:
        with (
            tc.tile_pool(name="const", bufs=1) as cp,
            tc.tile_pool(name="wstream", bufs=3) as wp,
            tc.tile_pool(name="istream", bufs=3) as ip,
            tc.tile_pool(name="gat", bufs=3) as gp,
            tc.tile_pool(name="prodp", bufs=2) as pp,
            tc.tile_pool(name="tmpp", bufs=2) as tp,
        ):
            bias_t = cp.tile([P, VPP * CHANNEL], mybir.dt.float32)
            acc_t = cp.tile([P, VPP * CHANNEL], mybir.dt.float32)
            out_t = cp.tile([P, VPP * CHANNEL], mybir.dt.float32)
            nc.sync.dma_start(out=bias_t[:], in_=b_d.ap())

            qn = 0
            for st in range(VPP):
                g_t = gp.tile([P, R * E], mybir.dt.bfloat16, tag="g")
                w_t = wp.tile([P, R * U], mybir.dt.bfloat16, tag="w")
                idx_t = ip.tile([P, NINST * (NPI // 16)], mybir.dt.int16,
                                tag="idx")
                nc.sync.dma_start(out=w_t[:], in_=w_ap[st])
                a = idx_ap[st]
                bcast = bass.AP(a.tensor, a.offset,
                                [[0, P // 16]] + list(a.ap))
                nc.sync.dma_start(out=idx_t[:], in_=bcast)
                for n in range(NINST):
                    base = n * SPI * E
                    out_ap = g_t[:, base:base + SPI * E].rearrange(
                        "p (q e) -> p q e", e=E)
                    _dma_gather_raw(
                        nc.gpsimd,
                        out_ap=out_ap,
                        in_ap=tab_ap[:, :E],
                        idxs_ap=idx_t[:, n * (NPI // 16):(n + 1) * (NPI // 16)],
                        num_idxs=NPI,
                        elem_size=E,
                        elem_step=PITCH,
                        queue_num=qn % NQUEUES,
                    )
                    qn += 1
                # products: [p, slot, sub, ch] = g * w (w bcast over ch)
                prod_t = pp.tile([P, R * E], mybir.dt.bfloat16, tag="prod")
                g4 = g_t[:].rearrange("p (s u c) -> p s u c", u=U, c=CHANNEL)
                wb = w_t[:].rearrange("p (s u) -> p s u", u=U).to_broadcast(
                    [P, R, U, CHANNEL])
                nc.vector.tensor_tensor(
                    out=prod_t[:].rearrange("p (s u c) -> p s u c",
                                            u=U, c=CHANNEL),
                    in0=g4, in1=wb, op=mybir.AluOpType.mult)
                # reduce over sub: [p, slot, ch, sub] -> [p, slot*ch]
                tmp_t = tp.tile([P, R * CHANNEL], mybir.dt.float32, tag="tmp")
                nc.vector.tensor_reduce(
                    out=tmp_t[:].rearrange("p (s c o) -> p s c o", c=CHANNEL,
                                           o=1),
                    in_=prod_t[:].rearrange("p (s u c) -> p s c u",
                                            u=U, c=CHANNEL),
                    axis=mybir.AxisListType.X, op=mybir.AluOpType.add)
                # reduce over slot: [p, ch, slot] -> acc[:, st*8:(st+1)*8]
                acc_sl = acc_t[:, st * CHANNEL:(st + 1) * CHANNEL]
                nc.vector.tensor_reduce(
                    out=acc_sl.rearrange("p (c o) -> p c o", o=1),
                    in_=tmp_t[:].rearrange("p (s c) -> p c s", c=CHANNEL),
                    axis=mybir.AxisListType.X, op=mybir.AluOpType.add)
            nc.vector.tensor_tensor(out=out_t[:], in0=acc_t[:], in1=bias_t[:],
                                    op=mybir.AluOpType.add)
            nc.sync.dma_start(out=out_d.ap(), in_=out_t[:])

    nc.compile()
    if hw:
        nc.m = get_hw_module(nc.m)
    return nc


class _Runner:
    """Compile once, execute the SPMD module on 8 cores via PJRT."""

    def __init__(self, nc, n_cores):
        import jax
        from jax.sharding import Mesh, PartitionSpec
        from jax.experimental.shard_map import shard_map
        from concourse.bass2jax import (_bass_exec_p, partition_id_tensor,
                                        install_neuronx_cc_hook)

        install_neuronx_cc_hook()
        self.jax = jax
        self.n_cores = n_cores
        in_names, out_names, out_avals = [], [], []
        pname = nc.partition_id_tensor.name if nc.partition_id_tensor else None
        for alloc in nc.m.functions[0].allocations:
            if not isinstance(alloc, mybir.MemoryLocationSet):
                continue
            name = alloc.memorylocations[0].name
            if alloc.kind == "ExternalInput":
                if name != pname:
                    in_names.append(name)
            elif alloc.kind == "ExternalOutput":
                out_names.append(name)
                out_avals.append(jax.core.ShapedArray(
                    tuple(alloc.tensor_shape), mybir.dt.np(alloc.dtype)))
        self.in_names, self.out_names, self.out_avals = in_names, out_names, out_avals
        all_in = list(in_names) + list(out_names) + ([pname] if pname else [])

        def _body(*args):
            operands = list(args)
            if pname is not None:
                operands.append(partition_id_tensor())
            return tuple(_bass_exec_p.bind(
                *operands, out_avals=tuple(out_avals), in_names=tuple(all_in),
                out_names=tuple(out_names), lowering_input_output_aliases=(),
                sim_require_finite=True, sim_require_nnan=True, nc=nc))

        devices = jax.devices()[:n_cores]
        self.mesh = Mesh(np.asarray(devices), ("core",))
        nin = len(in_names) + len(out_names)
        self.fn = jax.jit(
            shard_map(_body, mesh=self.mesh,
                      in_specs=(PartitionSpec("core"),) * nin,
                      out_specs=(PartitionSpec("core"),) * len(out_names),
                      check_rep=False),
            keep_unused=True)
        self._dev_in = None

    def set_inputs(self, in_maps):
        import jax
        from jax.sharding import NamedSharding, PartitionSpec
        sh = NamedSharding(self.mesh, PartitionSpec("core"))
        n = self.n_cores
        cat = [np.concatenate([np.asarray(in_maps[c][nm]) for c in range(n)], axis=0)
               for nm in self.in_names]
        zeros = [np.zeros((n * a.shape[0], *a.shape[1:]), a.dtype)
                 for a in self.out_avals]
        self._dev_in = [jax.device_put(x, sh) for x in cat + zeros]

    def run(self):
        outs = self.fn(*self._dev_in)
        self.jax.block_until_ready(outs)
        return outs

    def outputs_np(self, outs):
        n = self.n_cores
        return [
            {nm: np.asarray(outs[i]).reshape(n, *self.out_avals[i].shape)[c]
             for i, nm in enumerate(self.out_names)}
            for c in range(n)
        ]


_CACHE = {}


def _get_runner():
    if "k4" not in _CACHE:
        nc = _build_module()
        _CACHE["k4"] = _Runner(nc, NCORES)
    return _CACHE["k4"]


def prepare(x, weight, bias, indices):
    """Host-side marshalling: shard + build per-core device arrays."""
    x = np.asarray(x, np.float32).reshape(CHANNEL, V)
    weight = np.asarray(weight, np.float32).reshape(NVOX, R)
    bias = np.asarray(bias, np.float32).reshape(NVOX)
    indices = np.asarray(indices).astype(np.int64).reshape(NVOX, R)

    x_tab = np.zeros((ROWS, PITCH), np.float32)
    x_tab[:, :SUB * CHANNEL] = x.T.reshape(ROWS, SUB * CHANNEL)
    x_tab = x_tab.astype(ml_dtypes.bfloat16)

    in_maps = []
    for m in range(NCORES):
        sl = slice(m * VPC, (m + 1) * VPC)
        in_maps.append(_prep_core(x_tab, weight[sl], indices[sl], bias[sl]))
    return in_maps


def _sim_core(in_map):
    """Numpy emulation of the device program for one core (layout check)."""
    tab = in_map["tab"].astype(np.float32)            # [ROWS, PITCH]
    acc = np.zeros((P, VPP, CHANNEL), np.float32)
    for st in range(VPP):
        idxs = in_map["idx"][st]                      # [16, NINST*64]
        w = in_map["wts"][st].astype(np.float32)      # [P, R*U]
        g = np.empty((P, R, E), np.float32)
        for n in range(NINST):
            lst = idxs[:, n * 64:(n + 1) * 64].transpose(1, 0).reshape(NPI)
            gath = tab[lst.astype(np.int64), :E]      # [NPI, 24]
            g[:, n * SPI:(n + 1) * SPI] = gath.reshape(
                SPI, P, E).transpose(1, 0, 2)
        prod = (g.reshape(P, R, U, CHANNEL)
                * w.reshape(P, R, U, 1)).astype(
                    ml_dtypes.bfloat16).astype(np.float32)
        acc[:, st] = prod.sum(axis=(1, 2))
    acc += in_map["biasx"].reshape(P, VPP, CHANNEL)
    return acc.reshape(P, VPP * CHANNEL)


def kernel(x, weight, bias, indices):
    in_maps = prepare(x, weight, bias, indices)
    runner = _get_runner()
    runner.set_inputs(in_maps)
    outs = runner.run()
    per_core = runner.outputs_np(outs)
    full = np.empty((1, CHANNEL, NVOX), np.float32)
    for m in range(NCORES):
        o = per_core[m]["out"].reshape(P, VPP, CHANNEL)
        full[0, :, m * VPC:(m + 1) * VPC] = o.transpose(2, 0, 1).reshape(
            CHANNEL, VPC)
    return full.reshape(1, CHANNEL, NVX, NVY)


# revision 8
# speedup vs baseline: 1.1640x; 1.1640x over previous
"""Trainium2 Bass kernel for nn_BackProjNet (segment_reduce), triple-row table.

out[c, v] = (sum_r x[c, idx[v, r]] * w[v, r]) * SCALE + bias[v]

Strategy (8 NeuronCores, voxel-sharded):
  - Each core owns 8192 voxels (voxel v -> partition p = v//64, layer jj = v%64).
  - The sinogram table packs THREE positions per 256B row (row j holds
    x[:, 3j], x[:, 3j+1], x[:, 3j+2] as 24 bf16 = 48B payload), so all
    92160 positions fit in 30720 rows and gather indices stay int16 with
    NO position coloring and NO per-(voxel,color) padding: every voxel is
    exactly 360 gather slots (4.8% fewer SWDGE indices than the colored
    layout, and the kernel is descriptor-generation bound).
  - Gathers run as InstDMAGatherAnt (48B payload, 256B row stride), 1024
    indices per instruction on 4 SWDGE queues, 45 instructions per
    voxel-layer stage.
  - DVE multiplies by a host-packed masked weight stream (w at the ray's
    sub-position, 0 elsewhere, broadcast over the 8 channels) and reduces
    the 360x3 slot-sub grid per voxel; bias is added once at the end.
"""

import os
import sys

import numpy as np
import ml_dtypes

for _p in ("/opt/trn_rl_repo", "/root/.axon_site/_ro/trn_rl_repo"):
    if _p not in sys.path:
        sys.path.append(_p)

import concourse.bass as bass
import concourse.bacc as bacc
import concourse.mybir as mybir
import concourse.tile as tile
from concourse import ap_utils
from concourse._compat import exact_div
from concourse.bass import round_up_to_multiple
from concourse.bass_interp import get_hw_module

# geometry (must match reference.py)
CHANNEL = 8
NVX, NVY = 256, 256
VIEWS, EXTENT = 180, 2
NDETU = 512
SCALE = (2.0 * np.pi - 0.0) / (2.0 * VIEWS * EXTENT)

NCORES = 8
P = 128
V = VIEWS * NDETU          # 92160 sinogram positions
R = VIEWS * EXTENT         # 360 rays per voxel
NVOX = NVX * NVY           # 65536 voxels
VPC = NVOX // NCORES       # 8192 voxels per core
VPP = VPC // P             # 64 voxels per partition ("stages")
SUB = 3                    # positions per table row
ROWS = V // SUB            # 30720 table rows (int16-safe)
U = 3                      # sub slots per gathered row
E = U * CHANNEL            # 24 payload values per row (48B)
PITCH = 128                # bf16 per table row (256B)
NPI = 1024                 # indices per gather instruction (ucode limit)
SPI = NPI // P             # 8 slots per partition per instruction
NINST = R // SPI           # 45 gather instructions per stage
NQUEUES = int(os.environ.get("KQ", "4"))


def _dma_gather_raw(gpsimd, out_ap, in_ap, idxs_ap, num_idxs, elem_size,
                    elem_step, queue_num):
    """bass.dma_gather without the elem_size%256 restriction (the 256B
    constraint is on the row stride, which we satisfy with PITCH=128 bf16)."""
    self = gpsimd
    assert idxs_ap.dtype == mybir.dt.int16
    assert in_ap.space == bass.MemorySpace.DRAM
    assert in_ap.dtype == out_ap.dtype
    assert idxs_ap.space == bass.MemorySpace.SBUF
    assert out_ap.space == bass.MemorySpace.SBUF
    assert ap_utils.ap_is_contiguous(out_ap.ap[1:])
    assert ap_utils.ap_is_contiguous(idxs_ap.ap[1:])
    assert in_ap.ap[-1][1] == out_ap.ap[-1][1] == elem_size
    assert out_ap.ap[0][1] * out_ap.ap[1][1] == round_up_to_multiple(num_idxs, 128)
    assert in_ap.ap[0][0] == elem_step
    stride_bytes = elem_step * mybir.dt.size(in_ap.dtype)
    stride_bytes_256 = exact_div(stride_bytes, 256)
    _in_ap = self.lower_ap_dma(in_ap, for_custom_bir_dma=True)
    _idxs_ap = self.lower_ap(idxs_ap)
    _out_ap = self.lower_ap(out_ap)
    return self.add_instruction(
        mybir.InstDMAGatherAnt(
            name=self.bass.get_next_instruction_name(),
            ins=[*_in_ap, _idxs_ap, self.lower_val_access(self.to_reg(num_idxs))],
            outs=[_out_ap],
            transpose=False,
            num_idxs=num_idxs,
            elem_size=elem_size,
            stride_bytes_256=stride_bytes_256,
            gen_mode=0,
            single_packet=bool(int(os.environ.get("KSP", "1"))),
            queue_num=queue_num,
            sbuf_tokens_per_rank=0,
            sbuf_free_dim_per_rank=0,
            sbuf_free_dim_pad_per_rank=0,
            sbuf_byte_offset=0,
        )
    )


# ---------------------------------------------------------------- packing

def _prep_core(x_tab, w2, idx2, bias_m):
    """Build one core's device arrays.

    x_tab: shared table [ROWS, PITCH] bf16.  idx2/w2: [VPC, R].
    """
    rows = (idx2 // SUB).astype(np.int16)             # [VPC, R]
    subs = (idx2 % SUB).astype(np.int64)

    # per-partition voxel-major slot stream: partition p = voxel // VPP
    arr = rows.reshape(P, VPP, NINST, SPI)            # [p, stage, n, q]
    # instruction idx list i = q*128 + p, wrapped over 16 partitions
    a = arr.transpose(1, 2, 3, 0).reshape(VPP, NINST, NPI)
    idx_dev = np.ascontiguousarray(
        a.reshape(VPP, NINST, NPI // 16, 16).transpose(0, 1, 3, 2)
        .transpose(0, 2, 1, 3).reshape(VPP, 16, NINST * (NPI // 16)))

    w24 = np.zeros((VPC, R, U), np.float32)
    vv = np.repeat(np.arange(VPC), R)
    rr = np.tile(np.arange(R), VPC)
    w24[vv, rr, subs.ravel()] = (w2 * np.float32(SCALE)).ravel()
    w_dev = np.ascontiguousarray(
        w24.reshape(P, VPP, R * U).transpose(1, 0, 2)).astype(
            ml_dtypes.bfloat16)                       # [VPP, P, 1440]

    bias_dev = np.ascontiguousarray(
        np.repeat(bias_m.reshape(P, VPP), CHANNEL).reshape(P, VPP * CHANNEL))

    return dict(tab=x_tab, idx=idx_dev, wts=w_dev, biasx=bias_dev)


# ---------------------------------------------------------------- module

def _build_module(hw=True):
    nc = bacc.Bacc(
        "TRN2",
        target_bir_lowering=False,
        debug=False,
        num_devices=NCORES,
        dynamic_dma_scratch_size=16384,
        num_swdge_queues=NQUEUES,
    )
    tab_d = nc.dram_tensor("tab", [ROWS, PITCH], mybir.dt.bfloat16,
                           kind="ExternalInput")
    idx_d = nc.dram_tensor("idx", [VPP, 16, NINST * (NPI // 16)],
                           mybir.dt.int16, kind="ExternalInput")
    w_d = nc.dram_tensor("wts", [VPP, P, R * U], mybir.dt.bfloat16,
                         kind="ExternalInput")
    b_d = nc.dram_tensor("biasx", [P, VPP * CHANNEL], mybir.dt.float32,
                         kind="ExternalInput")
    out_d = nc.dram_tensor("out", [P, VPP * CHANNEL], mybir.dt.float32,
                           kind="ExternalOutput")

    tab_ap = tab_d.ap()
    idx_ap = idx_d.ap()
    w_ap = w_d.ap()

    with tile.TileContext(nc) as tc:
        with (
            tc.tile_pool(name="const", bufs=1) as cp,
            tc.tile_pool(name="wstream", bufs=3) as wp,
            tc.tile_pool(name="istream", bufs=3) as ip,
            tc.tile_pool(name="gat", bufs=3) as gp,
            tc.tile_pool(name="prodp", bufs=2) as pp,
            tc.tile_pool(name="tmpp", bufs=2) as tp,
        ):
            bias_t = cp.tile([P, VPP * CHANNEL], mybir.dt.float32)
            acc_t = cp.tile([P, VPP * CHANNEL], mybir.dt.float32)
            out_t = cp.tile([P, VPP * CHANNEL], mybir.dt.float32)
            nc.sync.dma_start(out=bias_t[:], in_=b_d.ap())

            qn = 0
            for st in range(VPP):
                g_t = gp.tile([P, R * E], mybir.dt.bfloat16, tag="g")
                w_t = wp.tile([P, R * U], mybir.dt.bfloat16, tag="w")
                idx_t = ip.tile([P, NINST * (NPI // 16)], mybir.dt.int16,
                                tag="idx")
                nc.sync.dma_start(out=w_t[:], in_=w_ap[st])
                a = idx_ap[st]
                bcast = bass.AP(a.tensor, a.offset,
                                [[0, P // 16]] + list(a.ap))
                nc.sync.dma_start(out=idx_t[:], in_=bcast)
                for n in range(NINST):
                    base = n * SPI * E
                    out_ap = g_t[:, base:base + SPI * E].rearrange(
                        "p (q e) -> p q e", e=E)
                    _dma_gather_raw(
                        nc.gpsimd,
                        out_ap=out_ap,
                        in_ap=tab_ap[:, :E],
                        idxs_ap=idx_t[:, n * (NPI // 16):(n + 1) * (NPI // 16)],
                        num_idxs=NPI,
                        elem_size=E,
                        elem_step=PITCH,
                        queue_num=qn % NQUEUES,
                    )
                    qn += 1
                # products: [p, slot, sub, ch] = g * w (w bcast over ch)
                prod_t = pp.tile([P, R * E], mybir.dt.bfloat16, tag="prod")
                g4 = g_t[:].rearrange("p (s u c) -> p s u c", u=U, c=CHANNEL)
                wb = w_t[:].rearrange("p (s u) -> p s u", u=U).to_broadcast(
                    [P, R, U, CHANNEL])
                nc.vector.tensor_tensor(
                    out=prod_t[:].rearrange("p (s u c) -> p s u c",
                                            u=U, c=CHANNEL),
                    in0=g4, in1=wb, op=mybir.AluOpType.mult)
                # reduce over sub: [p, slot, ch, sub] -> [p, slot*ch]
                tmp_t = tp.tile([P, R * CHANNEL], mybir.dt.float32, tag="tmp")
                nc.vector.tensor_reduce(
                    out=tmp_t[:].rearrange("p (s c o) -> p s c o", c=CHANNEL,
                                           o=1),
                    in_=prod_t[:].rearrange("p (s u c) -> p s c u",
                                            u=U, c=CHANNEL),
                    axis=mybir.AxisListType.X, op=mybir.AluOpType.add)
                # reduce over slot: [p, ch, slot] -> acc[:, st*8:(st+1)*8]
                acc_sl = acc_t[:, st * CHANNEL:(st + 1) * CHANNEL]
                nc.vector.tensor_reduce(
                    out=acc_sl.rearrange("p (c o) -> p c o", o=1),
                    in_=tmp_t[:].rearrange("p (s c) -> p c s", c=CHANNEL),
                    axis=mybir.AxisListType.X, op=mybir.AluOpType.add)
            nc.vector.tensor_tensor(out=out_t[:], in0=acc_t[:], in1=bias_t[:],
                                    op=mybir.AluOpType.add)
            nc.sync.dma_start(out=out_d.ap(), in_=out_t[:])

    nc.compile()
    if hw:
        nc.m = get_hw_module(nc.m)
    return nc


class _Runner:
    """Compile once, execute the SPMD module on 8 cores via PJRT."""

    def __init__(self, nc, n_cores):
        import jax
        from jax.sharding import Mesh, PartitionSpec
        from jax.experimental.shard_map import shard_map
        from concourse.bass2jax import (_bass_exec_p, partition_id_tensor,
                                        install_neuronx_cc_hook)

        install_neuronx_cc_hook()
        self.jax = jax
        self.n_cores = n_cores
        in_names, out_names, out_avals = [], [], []
        pname = nc.partition_id_tensor.name if nc.partition_id_tensor else None
        for alloc in nc.m.functions[0].allocations:
            if not isinstance(alloc, mybir.MemoryLocationSet):
                continue
            name = alloc.memorylocations[0].name
            if alloc.kind == "ExternalInput":
                if name != pname:
                    in_names.append(name)
            elif alloc.kind == "ExternalOutput":
                out_names.append(name)
                out_avals.append(jax.core.ShapedArray(
                    tuple(alloc.tensor_shape), mybir.dt.np(alloc.dtype)))
        self.in_names, self.out_names, self.out_avals = in_names, out_names, out_avals
        all_in = list(in_names) + list(out_names) + ([pname] if pname else [])

        def _body(*args):
            operands = list(args)
            if pname is not None:
                operands.append(partition_id_tensor())
            return tuple(_bass_exec_p.bind(
                *operands, out_avals=tuple(out_avals), in_names=tuple(all_in),
                out_names=tuple(out_names), lowering_input_output_aliases=(),
                sim_require_finite=True, sim_require_nnan=True, nc=nc))

        devices = jax.devices()[:n_cores]
        self.mesh = Mesh(np.asarray(devices), ("core",))
        nin = len(in_names) + len(out_names)
        self.fn = jax.jit(
            shard_map(_body, mesh=self.mesh,
                      in_specs=(PartitionSpec("core"),) * nin,
                      out_specs=(PartitionSpec("core"),) * len(out_names),
                      check_rep=False),
            keep_unused=True)
        self._dev_in = None

    def set_inputs(self, in_maps):
        import jax
        from jax.sharding import NamedSharding, PartitionSpec
        sh = NamedSharding(self.mesh, PartitionSpec("core"))
        n = self.n_cores
        cat = [np.concatenate([np.asarray(in_maps[c][nm]) for c in range(n)], axis=0)
               for nm in self.in_names]
        zeros = [np.zeros((n * a.shape[0], *a.shape[1:]), a.dtype)
                 for a in self.out_avals]
        self._dev_in = [jax.device_put(x, sh) for x in cat + zeros]

    def run(self):
        outs = self.fn(*self._dev_in)
        self.jax.block_until_ready(outs)
        return outs

    def outputs_np(self, outs):
        n = self.n_cores
        return [
            {nm: np.asarray(outs[i]).reshape(n, *self.out_avals[i].shape)[c]
             for i, nm in enumerate(self.out_names)}
            for c in range(n)
        ]


_CACHE = {}


def _get_runner():
    if "k4" not in _CACHE:
        nc = _build_module()
        _CACHE["k4"] = _Runner(nc, NCORES)
    return _CACHE["k4"]


def prepare(x, weight, bias, indices):
    """Host-side marshalling: shard + build per-core device arrays."""
    x = np.asarray(x, np.float32).reshape(CHANNEL, V)
    weight = np.asarray(weight, np.float32).reshape(NVOX, R)
    bias = np.asarray(bias, np.float32).reshape(NVOX)
    indices = np.asarray(indices).astype(np.int64).reshape(NVOX, R)

    x_tab = np.zeros((ROWS, PITCH), np.float32)
    x_tab[:, :SUB * CHANNEL] = x.T.reshape(ROWS, SUB * CHANNEL)
    x_tab = x_tab.astype(ml_dtypes.bfloat16)

    in_maps = []
    for m in range(NCORES):
        sl = slice(m * VPC, (m + 1) * VPC)
        in_maps.append(_prep_core(x_tab, weight[sl], indices[sl], bias[sl]))
    return in_maps


def _sim_core(in_map):
    """Numpy emulation of the device program for one core (layout check)."""
    tab = in_map["tab"].astype(np.float32)            # [ROWS, PITCH]
    acc = np.zeros((P, VPP, CHANNEL), np.float32)
    for st in range(VPP):
        idxs = in_map["idx"][st]                      # [16, NINST*64]
        w = in_map["wts"][st].astype(np.float32)      # [P, R*U]
        g = np.empty((P, R, E), np.float32)
        for n in range(NINST):
            lst = idxs[:, n * 64:(n + 1) * 64].transpose(1, 0).reshape(NPI)
            gath = tab[lst.astype(np.int64), :E]      # [NPI, 24]
            g[:, n * SPI:(n + 1) * SPI] = gath.reshape(
                SPI, P, E).transpose(1, 0, 2)
        prod = (g.reshape(P, R, U, CHANNEL)
                * w.reshape(P, R, U, 1)).astype(
                    ml_dtypes.bfloat16).astype(np.float32)
        acc[:, st] = prod.sum(axis=(1, 2))
    acc += in_map["biasx"].reshape(P, VPP, CHANNEL)
    return acc.reshape(P, VPP * CHANNEL)


def kernel(x, weight, bias, indices):
    in_maps = prepare(x, weight, bias, indices)
    runner = _get_runner()
    runner.set_inputs(in_maps)
    outs = runner.run()
    per_core = runner.outputs_np(outs)
    full = np.empty((1, CHANNEL, NVOX), np.float32)
    for m in range(NCORES):
        o = per_core[m]["out"].reshape(P, VPP, CHANNEL)
        full[0, :, m * VPC:(m + 1) * VPC] = o.transpose(2, 0, 1).reshape(
            CHANNEL, VPC)
    return full.reshape(1, CHANNEL, NVX, NVY)
